# revision 80
# baseline (speedup 1.0000x reference)
"""Multi-head attention kernel for Trainium2 (8 NeuronCores).

Problem: inputs query/key/value [2, 64, 64, 256] fp32, NHEAD=8, D=32.
reference: q,k,v -> [N=2, L=4096, H=8, D=32]; softmax(q.k^T/sqrt(D)) @ v.

Sharding: 16 (batch, head) pairs over 8 cores -> each core handles one
batch n = core//4 and two adjacent heads (2*hp, 2*hp+1), hp = core%4, so
its input slice is [4096, 64] contiguous channels.

v2 design (exp-bound -> dual-engine exp + HAM-aware PE stream):
  The kernel is exp-bound: 33.5M exps/core at 1 elem/cycle/lane on the
  ScalarE (1.2 GHz) is a ~220us floor by itself.  v2 splits the exp work
  between ScalarE (exact table exp, ~62% of groups) and VectorE
  (Schraudolph fast-exp: one tensor_scalar computes round(x*s1+s2) into
  int16 whose bits ARE bf16 2^(x*log2e); +-3% minimax log error that
  the shared softmax denominator largely cancels).  A greedy
  virtual-clock balancer assigns each (s-tile, head-pair) group to
  whichever engine is free sooner; boundary groups are pinned so each
  l-tile's epilogue lands at the front of the DVE queue.

  PE/HAM findings baked in: fp32/f32r matmuls and LDWEIGHTS do not
  count as activity for the PE's HAM clock gate, so an f32r MM1 stream
  pins the PE at 1.2 GHz forever.  Everything on the PE is therefore
  bf16 (MM1, MM2, transposes, the K=1 reciprocal broadcast is the one
  f32 exception), MM1s are emitted in cross-group pairs so four K=32
  row strips stream concurrently, and MM2s drain in ~10-pair
  back-to-back bursts -- a >=3.4us pure-matmul run is what trips HAM
  into the 2.4 GHz state.

  Epilogue (per l-tile): division-free Newton reciprocal from standard
  DVE ops only (custom-DVE ops don't compile on this walrus): seed via
  bit trick bf16_bits(1/d) ~= round(float(f32_bits(d))*-2^-16 + c)
  (Mitchell log2 + Schraudolph exp2 fused, +-6% minimax), one Newton
  step -> <=0.4%; K=1 PE matmuls broadcast 1/denom across partitions;
  one full-tile multiply + one full-tile 32x32 block transpose
  (garbage lanes riding along for free); output DMA via the gpsimd
  SWDGE queue.  The transpose prologue is folded into l-tile 0's
  emission with just-in-time DMA chunk loads.

Per-core layout (same as v1): Q^T,K^T [32*2 heads -> rows 0:64,
replicated 64:128, L cols] via PE transposes; V' [s,33]=[V|1] per head
(ones col = softmax denominator for free); S^T blocks [s=128, l=512]
per (s-tile, head); O'^T accum [33|33 at rows 0:33, 64:97, 512] per
l-tile.
"""

import numpy as np

L = 4096
D = 32
P = 128
NT = L // P            # 32 s-tiles per head
LT = 512               # l-tile width
N_LT = L // LT         # 8 l-tiles
TEMP = 1.0 / np.sqrt(np.float32(D))

# Schraudolph fast-exp constants (bf16 bit trick, round-to-nearest i16):
#   bits_i16 = round(x * TEMP*log2(e)*128 + (16256 - 5.51))
# 16256 = 127<<7 (bf16 exponent bias); 5.51 = 128*0.0861/2 is the
# minimax centering of the piecewise-linear log2 error (+-3.0% rel).
FEXP_S1 = float(TEMP * np.log2(np.e) * 128.0)
FEXP_S2 = float(16256.0 - 5.51)

_CACHE = {}


_MAXW = 1  # walrus codegen in this container allows 1 sem wait per instruction


def _split_waits_json(bir_json: bytes) -> bytes:
    """Rewrite BIR so no instruction carries more than _MAXW sem waits:
    excess waits move to EventSemaphore carrier instructions inserted
    immediately before, on the same engine (identical blocking semantics)."""
    import json

    m = json.loads(bir_json)
    ctr = 0
    for fn in m.get("functions", []):
        for blk in fn.get("blocks", []):
            out = []
            changed = False
            for ins in blk.get("instructions", []):
                si = ins.get("sync_info")
                waits = si.get("on_wait") if si else None
                if waits and len(waits) > _MAXW:
                    changed = True
                    excess = waits[: -_MAXW]
                    si["on_wait"] = waits[-_MAXW:]
                    for i in range(0, len(excess), _MAXW):
                        ctr += 1
                        out.append(
                            {
                                "debug": ins.get("debug", 0),
                                "engine": ins["engine"],
                                "ins": [],
                                "outs": [],
                                "name": f"EVW-{ctr}",
                                "opcode": "EventSemaphore",
                                "sync_info": {
                                    "on_wait": excess[i : i + _MAXW],
                                    "on_update": [],
                                },
                            }
                        )
                out.append(ins)
            if changed:
                blk["instructions"] = out
    return json.dumps(m).encode()


def _apply_drain_patch():
    """Hook compile_bir_kernel (both the native and the bass2jax/PJRT entry
    points) to run the wait-splitting BIR rewrite before walrus."""
    import concourse.bass_utils as bu

    if getattr(bu, "_ant_split_waits", False):
        return
    orig = bu.compile_bir_kernel

    def wrapped(bir_json, tmpdir, neff_name="file.neff"):
        return orig(_split_waits_json(bir_json), tmpdir, neff_name)

    bu.compile_bir_kernel = wrapped
    bu._ant_split_waits = True
    try:
        import concourse.bass2jax as b2j

        b2j.compile_bir_kernel = wrapped
    except ImportError:
        pass

    # (note: --enable-ldw-opt=true was tried here and is broken in this
    # walrus build -- visitInstLdweights codegen assertion)


def _build_v2(lag=12, no_dve_exp=False, mm1_bf16=True):
    """Dual-engine-exp build.  lag = MM2 groups held back (software
    pipeline depth, in gs=2 groups = s-tiles).  mm1_bf16: QK^T in bf16
    -- f32r matmuls do not count as activity for the PE's HAM clock
    gate, so an f32r MM1 stream pins the PE at 1.2 GHz; bf16 keeps it
    at 2.4 GHz (and costs ~0.5% weight noise, within budget)."""
    import concourse.bass as bass
    import concourse.mybir as mybir
    import concourse.tile as tile
    from concourse.masks import make_identity

    _apply_drain_patch()

    f32 = mybir.dt.float32
    f32r = mybir.dt.float32r
    bf16 = mybir.dt.bfloat16
    i16 = mybir.dt.int16
    i32 = mybir.dt.int32
    GW = 2 * LT  # exp group width (2 units = s-tile x {h0,h1})

    # virtual engine clocks (ns) for the greedy exp assignment
    ACT_G = 1110.0   # ACTIVATE [128,1024] psum->sbuf (measured)
    DVE_G = 1800.0   # tensor_scalar fast-exp [128,1024] psum->sbuf (measured)
    CAST_C = 840.0   # qt/kt cast [64,512] psum->sbuf f32r
    EPIL_C = 4400.0  # 3-op Newton recip + oc + mul + transpose

    nc = bass.Bass("TRN2", debug=False)
    q_d = nc.dram_tensor("q", [L, 64], f32, kind="ExternalInput")
    k_d = nc.dram_tensor("k", [L, 64], f32, kind="ExternalInput")
    v_d = nc.dram_tensor("v", [L, 64], f32, kind="ExternalInput")
    o_d = nc.dram_tensor("o", [L, 64], f32, kind="ExternalOutput")

    with tile.TileContext(nc) as tc:
        with (
            tc.tile_pool(name="const", bufs=1) as const_pool,
            tc.tile_pool(name="slab", bufs=1) as slab_pool,
            tc.tile_pool(name="persist", bufs=1) as persist_pool,
            tc.tile_pool(name="spsum", bufs=3, space="PSUM") as spsum,
            tc.tile_pool(name="apsum", bufs=2, space="PSUM") as apsum,
            tc.tile_pool(name="exps", bufs=3 + lag) as exps_pool,
            tc.tile_pool(name="epil", bufs=3) as epil_pool,
        ):
            ident = const_pool.tile([P, P], bf16)
            make_identity(nc, ident)

            ones_f = const_pool.tile([P, 64], f32)
            nc.vector.memset(ones_f, 1.0)
            # preload the exp ACT table while DMAs run (one tiny exp)
            warm = const_pool.tile([P, 8], f32)
            nc.scalar.activation(
                warm, ones_f[:, 0:8], mybir.ActivationFunctionType.Exp,
                scale=float(TEMP),
            )
            # K=1 broadcast matmul lhs: 1.0 at every partition, 32 cols.
            # bf16: the broadcast then streams at 1 cycle/row instead of
            # fp32's 4, and counts as HAM activity (f32r trips an ISA
            # dst-partition check here; fp32 is 4x slower and HAM-inert).
            onesb = persist_pool.tile([P, 32], f32)
            nc.vector.tensor_copy(out=onesb, in_=ones_f[:, 0:32])
            onesb_b = persist_pool.tile([P, 32], bf16)
            nc.vector.tensor_copy(out=onesb_b, in_=ones_f[:, 0:32])

            qs = slab_pool.tile([P, NT, 64], f32)
            ks = slab_pool.tile([P, NT, 64], f32)
            vs = slab_pool.tile([P, NT, 64], f32)
            # bf16 copies of the q/k slabs: the PE transposes then run as
            # single-pass bf16 matmuls (~7x faster than fp32 multi-pass,
            # and they count as HAM activity, keeping the PE at 2.4 GHz)
            qsb = slab_pool.tile([P, NT, 64], bf16)
            ksb = slab_pool.tile([P, NT, 64], bf16)
            q_ap = q_d.ap().rearrange("(t p) c -> p t c", p=P)
            k_ap = k_d.ap().rearrange("(t p) c -> p t c", p=P)
            v_ap = v_d.ap().rearrange("(t p) c -> p t c", p=P)

            def load_chunk(dst, src_ap, t0, n):
                ts_ = slice(t0, t0 + n)
                nc.sync.dma_start(out=dst[:, ts_, :], in_=src_ap[:, ts_, :])

            # V' = [v_h | 1] per head, per s-tile (bf16 for 1cyc/row MM2)
            vp = persist_pool.tile([P, NT, 66], bf16)

            def build_vp_chunk(j):  # j in 0..3, 8 s-tiles each (on GpSimd:
                # SBUF->SBUF with dtype convert, keeps the DVE free for exp)
                ts_ = slice(j * 8, j * 8 + 8)
                nc.gpsimd.tensor_copy(out=vp[:, ts_, 32:33], in_=ones_f[:, 0:8])
                nc.gpsimd.tensor_copy(out=vp[:, ts_, 65:66], in_=ones_f[:, 0:8])
                nc.gpsimd.tensor_copy(out=vp[:, ts_, 0:32], in_=vs[:, ts_, 0:32])
                nc.gpsimd.tensor_copy(out=vp[:, ts_, 33:65], in_=vs[:, ts_, 32:64])

            # Q^T, K^T [128, L]: rows 0:64 real (h0 d's, h1 d's),
            # rows 64:128 replicas for 4-way row-packed MM1.
            qkdt = bf16 if mm1_bf16 else f32r
            qt = persist_pool.tile([P, L], qkdt)
            kt = persist_pool.tile([P, L], qkdt)

            def transpose_group(dst, src, srcb, g):
                ts_ = slice(4 * g, 4 * g + 4)
                nc.vector.tensor_copy(out=srcb[:, ts_, :], in_=src[:, ts_, :])
                tp = spsum.tile([64, 4 * P], bf16, tag="sp")
                for j in range(4):
                    t = 4 * g + j
                    nc.tensor.transpose(
                        tp[:, j * P : (j + 1) * P], srcb[:, t, :], ident
                    )
                # PSUM->SBUF copy on the DVE: ScalarE is the binding
                # engine (~65% of exp groups), the DVE has idle slack
                nc.vector.tensor_copy(
                    out=dst[0:64, g * 512 : (g + 1) * 512], in_=tp
                )
                nc.sync.dma_start(
                    out=dst[64:128, g * 512 : (g + 1) * 512],
                    in_=dst[0:64, g * 512 : (g + 1) * 512],
                )

            # ---- main loop state --------------------------------------
            clocks = {"act": 0.0, "dve": 0.0}
            accum_by_lt = {}
            pend = []

            # Newton reciprocal constants: seed y0 via the bit trick
            #   bf16_bits(1/d) ~= round(float(f32_bits(d)) * -2^-16 + RC)
            # (Mitchell log2 + Schraudolph exp2 fused; log2 y0 - log2(1/d)
            # = c2 - h(m) - h(f) with h in [0, 0.0861], so c2 = 0.0861
            # centers it at 0 for minimax |err| <= 6.2%).  One Newton step
            # squares that to <=0.4%, and the 2.0012 in place of 2.0
            # centers the (always-low) quadratic residue.
            RC_S1 = float(-1.0 / 65536.0)
            RC_S2 = float(128.0 * (254.0 - 0.0861))

            def emit_epilogue(lt):
                lsl = slice(lt * LT, (lt + 1) * LT)
                accum = accum_by_lt.pop(lt)
                # reciprocal of the whole accum tile: only rows 32 / 96
                # (the denominators) are consumed; the rest is garbage
                # computed for free (same wall time, 512 elem/lane).
                # The LAST l-tile's epilogue is the kernel's critical
                # tail: use a bf16 reciprocal/broadcast (1 cyc/row matmul
                # vs fp32's 4) and put the oc copy on the by-then-idle
                # ScalarE so it overlaps the DVE's Newton ops.  The bf16
                # rounding (+-0.2% row scale) applies to 1/8 of rows.
                last = lt == N_LT - 1
                y0 = epil_pool.tile([P, LT], bf16, tag="y0")
                u = epil_pool.tile([P, LT], f32, tag="u")
                oc = epil_pool.tile([P, LT], f32, tag="oc")
                if last:
                    rec = epil_pool.tile([P, LT], bf16, tag="recb")
                    ones_mm = onesb_b
                else:
                    rec = epil_pool.tile([P, LT], f32, tag="rec")
                    ones_mm = onesb
                with nc.allow_low_precision(
                    reason="softmax denominators are O(4096); a 0.4% "
                    "reciprocal is far below the output tolerance"
                ):
                    nc.vector.tensor_scalar(
                        out=y0.bitcast(i16), in0=accum.bitcast(i32),
                        scalar1=RC_S1, scalar2=RC_S2,
                        op0=mybir.AluOpType.mult, op1=mybir.AluOpType.add,
                    )
                    # u = (d * -1) * y0 ; rec = (u + 2) * y0 = y0*(2 - d*y0)
                    nc.vector.scalar_tensor_tensor(
                        out=u, in0=accum, scalar=-1.0, in1=y0,
                        op0=mybir.AluOpType.mult, op1=mybir.AluOpType.mult,
                    )
                    # oc before rec: oc is the last accum reader, freeing
                    # the accum PSUM slot for the next l-tile's MM2s
                    if last:
                        nc.scalar.copy(out=oc, in_=accum)
                    else:
                        nc.vector.tensor_copy(out=oc, in_=accum)
                    nc.vector.scalar_tensor_tensor(
                        out=rec, in0=u, scalar=2.0012, in1=y0,
                        op0=mybir.AluOpType.add, op1=mybir.AluOpType.mult,
                    )
                # broadcast 1/denom to the 32 numerator rows of each head
                bc = apsum.tile([P, LT], f32, tag="accum")
                nc.tensor.matmul(
                    bc[0:32, :], ones_mm[32:33, :], rec[32:33, :],
                    start=True, stop=True, tile_position=(32, 0),
                )
                nc.tensor.matmul(
                    bc[64:96, :], ones_mm[96:97, :], rec[96:97, :],
                    start=True, stop=True, tile_position=(96, 64),
                )
                o_n = epil_pool.tile([P, LT], f32, tag="o_n")
                nc.vector.tensor_mul(o_n, oc, bc)
                # DVE 32x32 block transpose -> DRAM rows become 128B runs
                o_t = epil_pool.tile([P, LT], f32, tag="o_t")
                nc.vector.transpose(out=o_t, in_=o_n)
                clocks["dve"] += EPIL_C
                # outputs ride the gpsimd SWDGE queue to keep the sync
                # queue free for loads -- except the LAST l-tile, whose
                # DMA is on the kernel's critical tail: the sync HWDGE
                # queue is idle by then and has no SWDGE drain.
                dma_eng = nc.sync if lt == N_LT - 1 else nc.gpsimd
                for h in (0, 1):
                    dma_eng.dma_start(
                        out=o_d.ap()[lsl, 32 * h : 32 * h + 32].rearrange(
                            "(blk p) d -> p blk d", p=32
                        ),
                        in_=o_t[64 * h : 64 * h + 32, :].rearrange(
                            "p (blk d) -> p blk d", d=32
                        ),
                    )

            def flush_mm2(limit):
                while pend and len(pend) > limit:
                    lt, t, ex0, ex1 = pend.pop(0)
                    if lt not in accum_by_lt:
                        accum = apsum.tile([P, LT], f32, tag="accum")
                        accum_by_lt[lt] = accum
                    accum = accum_by_lt[lt]
                    st_f = dict(start=(t == 0), stop=(t == NT - 1))
                    nc.tensor.matmul(
                        accum[0:33, :], vp[:, t, 0:33], ex0,
                        tile_position=(0, 0), **st_f,
                    )
                    nc.tensor.matmul(
                        accum[64:97, :], vp[:, t, 33:66], ex1,
                        tile_position=(0, 64), **st_f,
                    )
                    if t == NT - 1:
                        emit_epilogue(lt)

            def emit_mm1(lt, t):
                lsl = slice(lt * LT, (lt + 1) * LT)
                sp = spsum.tile([P, GW], f32, tag="sp")
                for h in (0, 1):
                    # first pair of l-tile 0 sticks to the original rows
                    # (strips 0/1) so it does not wait for the row-
                    # replication DMA of transpose-group 0
                    st = 32 * (h if (lt == 0 and t < 2) else (2 * t + h) % 4)
                    nc.tensor.matmul(
                        sp[:, h * LT : (h + 1) * LT],
                        kt[st : st + 32, t * P : (t + 1) * P],
                        qt[st : st + 32, lsl],
                        start=True,
                        stop=True,
                        tile_position=(st, 0),
                    )
                return sp

            def emit_exp(lt, t, sp):
                ex = exps_pool.tile([P, GW], bf16, tag="ex")
                # Engine choice: greedy by virtual clocks, except around
                # l-tile boundaries.  The last two groups go to the DVE so
                # the epilogue (whose first op waits on MM2 of t=31) sits
                # directly behind its own exps in the DVE queue; the
                # groups feeding the boundary MM2 drain go to ACT so the
                # DVE queue stays clear and the drain is never exp-starved.
                if t >= NT - 1:
                    use_act = False
                elif t in (0, 1, 2, 3, NT - 4, NT - 3, NT - 2):
                    use_act = True
                else:
                    use_act = clocks["act"] + ACT_G <= clocks["dve"] + DVE_G
                if no_dve_exp or use_act:
                    clocks["act"] = max(clocks["act"], clocks["dve"] - 2500.0) + ACT_G
                    nc.scalar.activation(
                        ex, sp, mybir.ActivationFunctionType.Exp,
                        scale=float(TEMP),
                    )
                else:
                    clocks["dve"] = max(clocks["dve"], clocks["act"] - 2500.0) + DVE_G
                    with nc.allow_low_precision(
                        reason="Schraudolph bf16 fast-exp: zero-mean log "
                        "error, cancelled by the shared softmax denominator"
                    ):
                        nc.vector.tensor_scalar(
                            out=ex.bitcast(i16), in0=sp,
                            scalar1=FEXP_S1, scalar2=FEXP_S2,
                            op0=mybir.AluOpType.mult, op1=mybir.AluOpType.add,
                        )
                pend.append((lt, t, ex[:, 0:LT], ex[:, LT:GW]))
                # MM2s drain in bursts of ~lag pairs: a >=3.4us run of
                # back-to-back bf16 matmuls is what trips the PE's HAM
                # activity monitor into the 2.4 GHz state (the f32r MM1
                # stream alone never does).  At the l-tile end, drain
                # everything so the epilogue follows promptly.
                if lt == N_LT - 1 and t >= NT - 1 - lag:
                    flush_mm2(max(0, NT - 1 - t))
                elif t >= NT - 1:
                    flush_mm2(0)
                elif len(pend) > lag:
                    flush_mm2(1)

            # ---- emission ---------------------------------------------
            load_chunk(ks, k_ap, 0, 4)
            load_chunk(qs, q_ap, 0, 4)
            transpose_group(kt, ks, ksb, 0)
            transpose_group(qt, qs, qsb, 0)  # qt cols for l-tile 0
            clocks["dve"] += 2 * CAST_C

            def prologue_inserts(lt, t):
                # kt transpose groups run at double cadence (every 2
                # groups) so the K^T producer outruns the exp consumers
                # after the first few groups; qt groups are spread one
                # per l-tile (all transposes are bf16 and HAM-friendly).
                if lt == 0:
                    if t == 0:
                        load_chunk(ks, k_ap, 4, 4)
                        load_chunk(ks, k_ap, 8, 4)
                    if t % 2 == 0 and 2 <= t <= 14:
                        g = t // 2
                        transpose_group(kt, ks, ksb, g)
                        clocks["dve"] += CAST_C
                        if g + 2 < 8:
                            load_chunk(ks, k_ap, 4 * (g + 2), 4)
                    if t == 3:
                        load_chunk(vs, v_ap, 0, 8)
                        load_chunk(vs, v_ap, 8, 8)
                    if t == 7:
                        load_chunk(vs, v_ap, 16, 8)
                        load_chunk(vs, v_ap, 24, 8)
                    # vp chunk j must be emitted before the MM2 burst that
                    # first consumes s-tiles 8j.. (earliest at t = 8j+2)
                    if t in (5, 9, 13, 17):
                        build_vp_chunk((t - 5) // 4)
                if lt < N_LT - 1:
                    if t == 12:
                        load_chunk(qs, q_ap, 4 * (lt + 1), 4)
                    if t == 20:
                        transpose_group(qt, qs, qsb, lt + 1)
                        clocks["dve"] += CAST_C

            # MM1s emitted in cross-group pairs: (t, t+1) covers all
            # four 32-row strips, so four K=32 matmuls stream
            # concurrently in the PE array.  (Pre-emitting the next
            # l-tile's first MM1 pair ahead of the boundary drain was
            # tried and measured WORSE -- it perturbs the sp-ring
            # rotation; don't revisit.)
            for lt in range(N_LT):
                for t in range(0, NT, 2):
                    prologue_inserts(lt, t)
                    sp0 = emit_mm1(lt, t)
                    sp1 = emit_mm1(lt, t + 1)
                    emit_exp(lt, t, sp0)
                    prologue_inserts(lt, t + 1)
                    emit_exp(lt, t + 1, sp1)
            flush_mm2(0)
    return nc


def _build(mode="mixed"):
    """v1 build (kept as fallback).  mode: 'f32r' | 'bf16' | 'f32' | 'mixed'."""
    import concourse.bass as bass
    import concourse.mybir as mybir
    import concourse.tile as tile
    from concourse.masks import make_identity

    _apply_drain_patch()

    f32 = mybir.dt.float32
    if mode == "bf16":
        sdt = mybir.dt.bfloat16
        tdt = mybir.dt.bfloat16
        avdt = mybir.dt.bfloat16
    elif mode == "f32r":
        sdt = mybir.dt.float32r
        tdt = f32
        avdt = mybir.dt.float32r
    elif mode == "mixed":
        sdt = mybir.dt.float32r
        tdt = f32
        avdt = mybir.dt.bfloat16
    else:
        sdt = f32
        tdt = f32
        avdt = f32

    if mode == "bf16":
        gs, sp_bufs, lag = 2, 3, 9
    else:
        gs, sp_bufs, lag = 3, 2, 6

    nc = bass.Bass("TRN2", debug=False)
    q_d = nc.dram_tensor("q", [L, 64], f32, kind="ExternalInput")
    k_d = nc.dram_tensor("k", [L, 64], f32, kind="ExternalInput")
    v_d = nc.dram_tensor("v", [L, 64], f32, kind="ExternalInput")
    o_d = nc.dram_tensor("o", [L, 64], f32, kind="ExternalOutput")

    with tile.TileContext(nc) as tc:
        with (
            tc.tile_pool(name="const", bufs=1) as const_pool,
            tc.tile_pool(name="slab", bufs=1) as slab_pool,
            tc.tile_pool(name="persist", bufs=1) as persist_pool,
            tc.tile_pool(name="spsum", bufs=sp_bufs, space="PSUM") as spsum,
            tc.tile_pool(name="apsum", bufs=2, space="PSUM") as apsum,
            tc.tile_pool(name="exps", bufs=3 + lag) as exps_pool,
            tc.tile_pool(name="epil", bufs=4) as epil_pool,
        ):
            ident = const_pool.tile([P, P], tdt)
            make_identity(nc, ident)

            qs = slab_pool.tile([P, NT, 64], f32)
            ks = slab_pool.tile([P, NT, 64], f32)
            vs = slab_pool.tile([P, NT, 64], f32)
            for dst_t, src_t in ((qs, q_d), (ks, k_d), (vs, v_d)):
                src_ap = src_t.ap().rearrange("(t p) c -> p t c", p=P)
                for c4 in range(4):
                    ts_ = slice(c4 * 8, c4 * 8 + 8)
                    nc.sync.dma_start(out=dst_t[:, ts_, :], in_=src_ap[:, ts_, :])

            if mode == "bf16":
                qsb = slab_pool.tile([P, NT, 64], tdt)
                ksb = slab_pool.tile([P, NT, 64], tdt)
                for i in range(4):
                    s = slice(i * 8, i * 8 + 8)
                    nc.vector.tensor_copy(out=qsb[:, s, :], in_=qs[:, s, :])
                    nc.vector.tensor_copy(out=ksb[:, s, :], in_=ks[:, s, :])
            else:
                qsb, ksb = qs, ks

            ones_f = const_pool.tile([P, 64], f32)
            nc.vector.memset(ones_f, 1.0)
            vp = persist_pool.tile([P, NT, 66], avdt)
            nc.vector.tensor_copy(out=vp[:, :, 32:33], in_=ones_f[:, 0:NT])
            nc.vector.tensor_copy(out=vp[:, :, 65:66], in_=ones_f[:, 0:NT])
            for c4 in range(4):
                ts_ = slice(c4 * 8, c4 * 8 + 8)
                nc.vector.tensor_copy(
                    out=vp[:, ts_, 0:32], in_=vs[:, ts_, 0:32]
                )
                nc.vector.tensor_copy(
                    out=vp[:, ts_, 33:65], in_=vs[:, ts_, 32:64]
                )
            onesb = persist_pool.tile([P, 33], avdt)
            nc.vector.tensor_copy(out=onesb, in_=ones_f[:, 0:33])

            qt = persist_pool.tile([P, L], sdt)
            kt = persist_pool.tile([P, L], sdt)
            for dst, src in ((qt, qsb), (kt, ksb)):
                for g in range(NT // 4):
                    tp = spsum.tile([64, 4 * P], tdt, tag="sp")
                    for j in range(4):
                        t = 4 * g + j
                        nc.tensor.transpose(
                            tp[:, j * P : (j + 1) * P], src[:, t, :], ident
                        )
                    nc.vector.tensor_copy(
                        out=dst[0:64, g * 512 : (g + 1) * 512], in_=tp
                    )
                    nc.sync.dma_start(
                        out=dst[64:128, g * 512 : (g + 1) * 512],
                        in_=dst[0:64, g * 512 : (g + 1) * 512],
                    )

            units = [(t, h) for t in range(NT) for h in (0, 1)]
            groups = [units[i : i + gs] for i in range(0, len(units), gs)]

            accum_by_lt = {}
            ex_loc = {}
            pend = []

            def emit_epilogue(lt):
                lsl = slice(lt * LT, (lt + 1) * LT)
                accum = accum_by_lt.pop(lt)
                rec = epil_pool.tile([P, LT], avdt, tag="rec")
                with nc.allow_low_precision(
                    reason="softmax denominators are O(4096); rounding "
                    "the reciprocal to the matmul dtype is harmless"
                ):
                    nc.vector.reciprocal(
                        out=rec[32:33, :], in_=accum[32:33, :]
                    )
                    nc.vector.reciprocal(
                        out=rec[96:97, :], in_=accum[96:97, :]
                    )
                bc = apsum.tile([P, LT], f32, tag="accum")
                nc.tensor.matmul(
                    bc[0:32, :], onesb[32:33, 0:32], rec[32:33, :],
                    start=True, stop=True, tile_position=(32, 0),
                )
                nc.tensor.matmul(
                    bc[64:96, :], onesb[96:97, 0:32], rec[96:97, :],
                    start=True, stop=True, tile_position=(96, 64),
                )
                oc = epil_pool.tile([P, LT], f32, tag="oc")
                nc.vector.tensor_copy(out=oc, in_=accum)
                o_n = epil_pool.tile([P, LT], f32, tag="o_n")
                nc.vector.tensor_mul(o_n[0:32, :], oc[0:32, :], bc[0:32, :])
                nc.vector.tensor_mul(
                    o_n[64:96, :], oc[64:96, :], bc[64:96, :]
                )
                o_t = epil_pool.tile([P, LT], f32, tag="o_t")
                nc.vector.transpose(out=o_t[0:32, :], in_=o_n[0:32, :])
                nc.vector.transpose(out=o_t[64:96, :], in_=o_n[64:96, :])
                for h in (0, 1):
                    nc.sync.dma_start(
                        out=o_d.ap()[lsl, 32 * h : 32 * h + 32].rearrange(
                            "(blk p) d -> p blk d", p=32
                        ),
                        in_=o_t[64 * h : 64 * h + 32, :].rearrange(
                            "p (blk d) -> p blk d", d=32
                        ),
                    )

            def flush_mm2(limit):
                while pend and len(pend) > limit:
                    lt, t, ex0, ex1 = pend.pop(0)
                    if lt not in accum_by_lt:
                        accum = apsum.tile([P, LT], f32, tag="accum")
                        accum_by_lt[lt] = accum
                    accum = accum_by_lt[lt]
                    st_f = dict(start=(t == 0), stop=(t == NT - 1))
                    nc.tensor.matmul(
                        accum[0:33, :], vp[:, t, 0:33], ex0,
                        tile_position=(0, 0), **st_f,
                    )
                    nc.tensor.matmul(
                        accum[64:97, :], vp[:, t, 33:66], ex1,
                        tile_position=(0, 64), **st_f,
                    )
                    if t == NT - 1:
                        emit_epilogue(lt)

            def emit_group(lt, grp):
                lsl = slice(lt * LT, (lt + 1) * LT)
                w = len(grp) * LT
                sp = spsum.tile([P, gs * LT], f32, tag="sp")
                ex = exps_pool.tile([P, gs * LT], avdt, tag="ex")
                for j, (t, h) in enumerate(grp):
                    st = 32 * ((2 * t + h) % 4)
                    nc.tensor.matmul(
                        sp[:, j * LT : (j + 1) * LT],
                        kt[st : st + 32, t * P : (t + 1) * P],
                        qt[st : st + 32, lsl],
                        start=True,
                        stop=True,
                        tile_position=(st, 0),
                    )
                nc.scalar.activation(
                    ex[:, :w],
                    sp[:, :w],
                    mybir.ActivationFunctionType.Exp,
                    scale=float(TEMP),
                )
                for j, (t, h) in enumerate(grp):
                    ex_loc[(lt, t, h)] = ex[:, j * LT : (j + 1) * LT]
                while ex_loc:
                    klt, kt_ = min((a, b) for a, b, _ in ex_loc)
                    if (klt, kt_, 0) not in ex_loc or (klt, kt_, 1) not in ex_loc:
                        break
                    pend.append(
                        (
                            klt,
                            kt_,
                            ex_loc.pop((klt, kt_, 0)),
                            ex_loc.pop((klt, kt_, 1)),
                        )
                    )
                flush_mm2(lag)

            n_g = len(groups)
            for lt in range(N_LT):
                for gi, grp in enumerate(groups):
                    emit_group(lt, grp)
                    if lt == N_LT - 1 and gi > n_g - lag:
                        flush_mm2(max(0, n_g - 1 - gi))
            flush_mm2(0)
    return nc


def _get_nc(mode):
    if mode not in _CACHE:
        if mode == "v2":
            _CACHE[mode] = _build_v2()
        elif mode == "v2-noexp":
            _CACHE[mode] = _build_v2(no_dve_exp=True)
        else:
            _CACHE[mode] = _build(mode)
    return _CACHE[mode]


def kernel(query, key, value, mode="v2", trace=False):
    from concourse.bass_utils import run_bass_kernel_spmd

    q = np.ascontiguousarray(np.asarray(query, np.float32)).reshape(2, L, 256)
    k = np.ascontiguousarray(np.asarray(key, np.float32)).reshape(2, L, 256)
    v = np.ascontiguousarray(np.asarray(value, np.float32)).reshape(2, L, 256)

    nc = _get_nc(mode)
    in_maps = []
    for c in range(8):
        n, hp = divmod(c, 4)
        sl = slice(64 * hp, 64 * hp + 64)
        in_maps.append(
            {
                "q": np.ascontiguousarray(q[n, :, sl]),
                "k": np.ascontiguousarray(k[n, :, sl]),
                "v": np.ascontiguousarray(v[n, :, sl]),
            }
        )
    kwargs = {}
    if trace:
        kwargs = dict(trace=True)
    res = run_bass_kernel_spmd(nc, in_maps, core_ids=list(range(8)), **kwargs)
    out = np.zeros((2, L, 8, 32), np.float32)
    for c, r in enumerate(res.results):
        n, hp = divmod(c, 4)
        out[n, :, 2 * hp : 2 * hp + 2, :] = r["o"].reshape(L, 2, 32)
    if trace:
        return out, res
    return out
